# revision 3
# baseline (speedup 1.0000x reference)
"""Trainium2 Bass kernel for linear attention over external memory.

Computes out = x @ (keys^T @ vals) for
  x [4, 2048, 1024] f32, keys/vals [65536, 1024] f32.

Sharding across 8 NeuronCores: keys/vals sharded along the memory dim M
(8192 rows per core); each core computes a partial kv = keys_s^T @ vals_s
in two e-column halves (each half accumulated across the full k range in
8 PSUM banks), AllReduces each half in bf16 while other work proceeds,
then computes its token shard of x @ kv (x sharded by token, 1024 rows
per core).
"""

import numpy as np

# Problem shapes (hardcoded per contract).
B, S, D = 4, 2048, 1024
M = 65536
NCORES = 8
P = 128
T = (B * S) // NCORES          # 1024 tokens per core
KM = M // NCORES               # 8192 memory rows per core
C2 = 2                         # k-chunks per DMA tile
NT = KM // (P * C2)            # 32 two-chunk key/val tiles
DB = D // P                    # 8 d-blocks
HALF = D // 2                  # 512 e-columns per half
TCH = T // P                   # 8 token chunks

_CACHE = {}


def _build_nc():
    import concourse.bacc as bacc
    import concourse.tile as tile
    from concourse import mybir
    from concourse.masks import make_identity

    f32 = mybir.dt.float32
    bf16 = mybir.dt.bfloat16

    nc = bacc.Bacc("TRN2", target_bir_lowering=False, debug=False,
                   num_devices=NCORES)

    xs_d = nc.dram_tensor("xs", [T, D], f32, kind="ExternalInput")
    ks_d = nc.dram_tensor("ks", [KM, D], f32, kind="ExternalInput")
    vs_d = nc.dram_tensor("vs", [KM, D], f32, kind="ExternalInput")
    out_d = nc.dram_tensor("out", [T, D], f32, kind="ExternalOutput")

    # DRAM views with rows split as (tile, chunk, partition).
    ks_r = ks_d.ap().rearrange("(t c p) n -> t p c n", c=C2, p=P)
    vs_r = vs_d.ap().rearrange("(t c p) n -> t p c n", c=C2, p=P)
    xs_r = xs_d.ap().rearrange("(t c p) n -> t p c n", c=C2, p=P)
    NXT = T // (P * C2)  # 4 x tiles

    with tile.TileContext(nc) as tc:
        with (
            tc.tile_pool(name="const", bufs=1) as const,
            tc.tile_pool(name="keysbf", bufs=NT) as keysbf_pool,
            tc.tile_pool(name="kstage", bufs=2) as kstage,
            tc.tile_pool(name="vstage", bufs=2) as vstage,
            tc.tile_pool(name="vbfp", bufs=3) as vbfp,
            tc.tile_pool(name="xstage", bufs=1) as xstage,
            tc.tile_pool(name="xtp", bufs=DB) as xtp,
            tc.tile_pool(name="kvev", bufs=1) as kvev,
            tc.tile_pool(name="kvrp", bufs=1) as kvrp,
            tc.tile_pool(name="outp", bufs=2) as outp,
            tc.tile_pool(name="ps", bufs=8, space="PSUM") as ps,
            tc.tile_pool(name="dram", bufs=2, space="DRAM") as dram,
        ):
            ident = const.tile([P, P], f32)
            make_identity(nc, ident)

            # ---- x: load, PE-transpose, cast to bf16 (resident xT) ----
            # xT[j] holds x[:, j*128:(j+1)*128]^T as [d=128, t=1024] bf16.
            xT = [xtp.tile([P, T], bf16, name=f"xT{j}", tag="xT")
                  for j in range(DB)]
            for t in range(NXT):
                xf = xstage.tile([P, C2, D], f32, name="xf", tag="xf")
                nc.sync.dma_start(out=xf[:], in_=xs_r[t])
                for c in range(C2):
                    i = t * C2 + c  # token chunk index
                    for j in range(DB):
                        pst = ps.tile([P, P], f32, name="pst", tag="ps")
                        nc.tensor.transpose(
                            pst[:], xf[:, c, j * P:(j + 1) * P], ident[:])
                        nc.vector.tensor_copy(
                            out=xT[j][:, i * P:(i + 1) * P], in_=pst[:])

            # ---- keys: load + cast to bf16, fully resident ----
            # keys_bf[t] is [p=128, c=2, D] bf16 (rows t*256 + c*128 + p).
            keys_bf = []
            for t in range(NT):
                kf = kstage.tile([P, C2, D], f32, name="kf", tag="kf")
                nc.sync.dma_start(out=kf[:], in_=ks_r[t])
                kb = keysbf_pool.tile([P, C2, D], bf16, name=f"kb{t}",
                                      tag="kb")
                if t % 2 == 0:
                    nc.vector.tensor_copy(out=kb[:], in_=kf[:])
                else:
                    nc.scalar.activation(
                        kb[:], kf[:], mybir.ActivationFunctionType.Copy)
                keys_bf.append(kb)

            # ---- kv halves + AllReduce + stage 4 ----
            kvr = []  # reduced kv halves, bf16 [128, DB*HALF]
            ar_bufs = []
            for h in range(2):
                e0 = h * HALF
                kv_ps = [ps.tile([P, HALF], f32, name=f"kv{h}_{j}", tag="ps")
                         for j in range(DB)]
                for t in range(NT):
                    vf = vstage.tile([P, C2, HALF], f32, name="vf", tag="vf")
                    nc.sync.dma_start(
                        out=vf[:], in_=vs_r[t][:, :, e0:e0 + HALF])
                    vb = vbfp.tile([P, C2, HALF], bf16, name="vb", tag="vb")
                    if t % 2 == 0:
                        nc.scalar.activation(
                            vb[:], vf[:], mybir.ActivationFunctionType.Copy)
                    else:
                        nc.vector.tensor_copy(out=vb[:], in_=vf[:])
                    for c in range(C2):
                        first = (t == 0 and c == 0)
                        last = (t == NT - 1 and c == C2 - 1)
                        for j in range(DB):
                            nc.tensor.matmul(
                                kv_ps[j][:],
                                keys_bf[t][:, c, j * P:(j + 1) * P],
                                vb[:, c, :],
                                start=first, stop=last)

                # Evict psum -> sbuf bf16, DMA to bounce, AllReduce.
                kv_sb = kvev.tile([P, DB * HALF], bf16, name="kv_sb",
                                  tag="kvev")
                for j in range(DB):
                    nc.vector.tensor_copy(
                        out=kv_sb[:, j * HALF:(j + 1) * HALF],
                        in_=kv_ps[j][:])
                bounce_in = dram.tile([P, DB * HALF], bf16,
                                      name=f"bin{h}", tag="bin")
                bounce_out = dram.tile([P, DB * HALF], bf16,
                                       name=f"bout{h}", tag="bout",
                                       addr_space="Shared")
                nc.sync.dma_start(out=bounce_in[:], in_=kv_sb[:])
                nc.gpsimd.collective_compute(
                    "AllReduce",
                    mybir.AluOpType.add,
                    replica_groups=[list(range(NCORES))],
                    ins=[bounce_in.opt()],
                    outs=[bounce_out.opt()],
                )
                kvh = kvrp.tile([P, DB * HALF], bf16, name=f"kvr{h}",
                                tag="kvr")
                nc.sync.dma_start(out=kvh[:], in_=bounce_out[:])
                kvr.append(kvh)
                ar_bufs.append((bounce_in, bounce_out))

                # ---- stage 4 for this half: out[:, e0:e0+HALF] ----
                for i in range(TCH):
                    po = ps.tile([P, HALF], f32, name="po", tag="ps")
                    for j in range(DB):
                        nc.tensor.matmul(
                            po[:],
                            xT[j][:, i * P:(i + 1) * P],
                            kvh[:, j * HALF:(j + 1) * HALF],
                            start=(j == 0), stop=(j == DB - 1))
                    ob = outp.tile([P, HALF], f32, name="ob", tag="ob")
                    nc.scalar.activation(
                        ob[:], po[:], mybir.ActivationFunctionType.Copy)
                    nc.sync.dma_start(
                        out=out_d.ap()[i * P:(i + 1) * P, e0:e0 + HALF],
                        in_=ob[:])

    nc.compile()
    return nc


def _get_nc():
    if "nc" not in _CACHE:
        _CACHE["nc"] = _build_nc()
    return _CACHE["nc"]


def kernel(**inputs):
    from concourse.bass_utils import run_bass_kernel_spmd

    x = np.ascontiguousarray(np.asarray(inputs["x"], dtype=np.float32))
    keys = np.ascontiguousarray(np.asarray(inputs["keys"], dtype=np.float32))
    vals = np.ascontiguousarray(np.asarray(inputs["vals"], dtype=np.float32))
    xf = x.reshape(B * S, D)

    nc = _get_nc()
    in_maps = []
    for c in range(NCORES):
        in_maps.append({
            "xs": xf[c * T:(c + 1) * T],
            "ks": keys[c * KM:(c + 1) * KM],
            "vs": vals[c * KM:(c + 1) * KM],
        })
    res = run_bass_kernel_spmd(nc, in_maps, list(range(NCORES)))
    out = np.concatenate([res.results[c]["out"] for c in range(NCORES)],
                         axis=0)
    return out.reshape(B, S, D).astype(np.float32)


# revision 4
# speedup vs baseline: 1.3002x; 1.3002x over previous
"""Trainium2 Bass kernel for linear attention over external memory.

Computes out = x @ (keys^T @ vals) for
  x [4, 2048, 1024] f32, keys/vals [65536, 1024] f32.

Sharding across 8 NeuronCores: keys/vals sharded along the memory dim M
(8192 rows per core); each core computes a partial kv = keys_s^T @ vals_s,
AllReduces kv in bf16 (split in two column halves so the first AllReduce
overlaps remaining work), then computes its token shard of x @ kv
(x sharded by token, 1024 rows per core).

Stage 2 runs in float32r (TF32-like, full PE rate for moving dim >= 256)
directly on the DMA'd f32 data — no cast step. kv is accumulated in
PSUM per group of 8 k-chunks and drained into an SBUF f32 accumulator.
"""

import numpy as np

# Problem shapes (hardcoded per contract).
B, S, D = 4, 2048, 1024
M = 65536
NCORES = 8
P = 128
T = (B * S) // NCORES          # 1024 tokens per core
KM = M // NCORES               # 8192 memory rows per core
NC_ = KM // P                  # 64 k-chunks
G = 8                          # chunks per PSUM accumulation group
NG = NC_ // G                  # 8 groups
DB = D // P                    # 8 d-blocks
HALF = D // 2                  # 512
TCH = T // P                   # 8 token chunks

_CACHE = {}


def _build_nc():
    import concourse.bacc as bacc
    import concourse.tile as tile
    from concourse import mybir
    from concourse.masks import make_identity

    f32 = mybir.dt.float32
    f32r = mybir.dt.float32r
    bf16 = mybir.dt.bfloat16
    ACT_COPY = mybir.ActivationFunctionType.Copy

    nc = bacc.Bacc("TRN2", target_bir_lowering=False, debug=False,
                   num_devices=NCORES)

    xs_d = nc.dram_tensor("xs", [T, D], f32, kind="ExternalInput")
    ks_d = nc.dram_tensor("ks", [KM, D], f32r, kind="ExternalInput")
    vs_d = nc.dram_tensor("vs", [KM, D], f32r, kind="ExternalInput")
    out_d = nc.dram_tensor("out", [T, D], f32, kind="ExternalOutput")

    ks_r = ks_d.ap().rearrange("(c p) n -> c p n", p=P)   # [64, 128, 1024]
    vs_r = vs_d.ap().rearrange("(c p) n -> c p n", p=P)
    xs_r = xs_d.ap().rearrange("(c p) n -> c p n", p=P)   # [8, 128, 1024]

    with tile.TileContext(nc) as tc:
        with (
            tc.tile_pool(name="const", bufs=1) as const,
            tc.tile_pool(name="kfp", bufs=14) as kfp,
            tc.tile_pool(name="vfp", bufs=14) as vfp,
            tc.tile_pool(name="accp", bufs=2 * DB) as accp,
            tc.tile_pool(name="xstage", bufs=2) as xstage,
            tc.tile_pool(name="xtp", bufs=DB) as xtp,
            tc.tile_pool(name="kvio", bufs=2) as kvio,
            tc.tile_pool(name="outp", bufs=3) as outp,
            tc.tile_pool(name="ps", bufs=8, space="PSUM") as ps,
            tc.tile_pool(name="dram", bufs=4, space="DRAM") as dram,
        ):
            ident = const.tile([P, P], f32)
            make_identity(nc, ident)

            # kv accumulator: tile (h*DB+j) holds kv[j*128:(j+1)*128,
            # h*512:(h+1)*512] as [128, 512] f32.
            acc = [accp.tile([P, HALF], f32, name=f"acc{i}", tag="acc")
                   for i in range(2 * DB)]
            for i in range(2 * DB):
                nc.gpsimd.memset(acc[i][:], 0.0)

            # ---- stage 2: kv partial, grouped PSUM accumulation ----
            for g in range(NG):
                kf = []
                vf = []
                for c in range(G):
                    kt = kfp.tile([P, D], f32r, name="kt", tag="kt")
                    nc.sync.dma_start(out=kt[:], in_=ks_r[g * G + c])
                    vt = vfp.tile([P, D], f32r, name="vt", tag="vt")
                    nc.sync.dma_start(out=vt[:], in_=vs_r[g * G + c])
                    kf.append(kt)
                    vf.append(vt)
                for h in range(2):
                    e0 = h * HALF
                    pst = [ps.tile([P, HALF], f32, name=f"kv{h}_{j}",
                                   tag="ps") for j in range(DB)]
                    for c in range(G):
                        for j in range(DB):
                            nc.tensor.matmul(
                                pst[j][:],
                                kf[c][:, j * P:(j + 1) * P],
                                vf[c][:, e0:e0 + HALF],
                                start=(c == 0), stop=(c == G - 1))
                    for j in range(DB):
                        nc.vector.tensor_tensor(
                            out=acc[h * DB + j][:],
                            in0=pst[j][:],
                            in1=acc[h * DB + j][:],
                            op=mybir.AluOpType.add)

            # ---- AllReduce kv, split by column half ----
            kvr = []
            for h in range(2):
                kvev = kvio.tile([P, DB * HALF], bf16, name=f"kvev{h}",
                                 tag="kvio")
                for j in range(DB):
                    nc.scalar.activation(
                        kvev[:, j * HALF:(j + 1) * HALF],
                        acc[h * DB + j][:], ACT_COPY)
                bounce_in = dram.tile([P, DB * HALF], bf16,
                                      name=f"bin{h}", tag="bin")
                bounce_out = dram.tile([P, DB * HALF], bf16,
                                       name=f"bout{h}", tag="bout",
                                       addr_space="Shared")
                nc.gpsimd.dma_start(out=bounce_in[:], in_=kvev[:])
                nc.gpsimd.collective_compute(
                    "AllReduce",
                    mybir.AluOpType.add,
                    replica_groups=[list(range(NCORES))],
                    ins=[bounce_in.opt()],
                    outs=[bounce_out.opt()],
                )
                kvr.append(bounce_out)

            # ---- x: load, PE-transpose, cast to bf16 (fills AR wait) ----
            xT = [xtp.tile([P, T], bf16, name=f"xT{j}", tag="xT")
                  for j in range(DB)]
            for i in range(TCH):
                xf = xstage.tile([P, D], f32, name="xf", tag="xf")
                nc.sync.dma_start(out=xf[:], in_=xs_r[i])
                for j in range(DB):
                    pst = ps.tile([P, P], f32, name="pst", tag="ps")
                    nc.tensor.transpose(
                        pst[:], xf[:, j * P:(j + 1) * P], ident[:])
                    nc.vector.tensor_copy(
                        out=xT[j][:, i * P:(i + 1) * P], in_=pst[:])

            # ---- stage 4: out = x @ kv, per column half ----
            for h in range(2):
                kvh = kvio.tile([P, DB * HALF], bf16, name=f"kvr{h}",
                                tag="kvio")
                nc.gpsimd.dma_start(out=kvh[:], in_=kvr[h][:])
                for i in range(TCH):
                    po = ps.tile([P, HALF], f32, name="po", tag="ps")
                    for j in range(DB):
                        nc.tensor.matmul(
                            po[:],
                            xT[j][:, i * P:(i + 1) * P],
                            kvh[:, j * HALF:(j + 1) * HALF],
                            start=(j == 0), stop=(j == DB - 1))
                    ob = outp.tile([P, HALF], f32, name="ob", tag="ob")
                    nc.scalar.activation(ob[:], po[:], ACT_COPY)
                    nc.scalar.dma_start(
                        out=out_d.ap()[i * P:(i + 1) * P,
                                       h * HALF:(h + 1) * HALF],
                        in_=ob[:])

    nc.compile()
    return nc


def _get_nc():
    if "nc" not in _CACHE:
        _CACHE["nc"] = _build_nc()
    return _CACHE["nc"]


def kernel(**inputs):
    from concourse.bass_utils import run_bass_kernel_spmd

    x = np.ascontiguousarray(np.asarray(inputs["x"], dtype=np.float32))
    keys = np.ascontiguousarray(np.asarray(inputs["keys"], dtype=np.float32))
    vals = np.ascontiguousarray(np.asarray(inputs["vals"], dtype=np.float32))
    xf = x.reshape(B * S, D)

    nc = _get_nc()
    in_maps = []
    for c in range(NCORES):
        in_maps.append({
            "xs": xf[c * T:(c + 1) * T],
            "ks": keys[c * KM:(c + 1) * KM],
            "vs": vals[c * KM:(c + 1) * KM],
        })
    res = run_bass_kernel_spmd(nc, in_maps, list(range(NCORES)))
    out = np.concatenate([res.results[c]["out"] for c in range(NCORES)],
                         axis=0)
    return out.reshape(B, S, D).astype(np.float32)
